# revision 13
# baseline (speedup 1.0000x reference)
"""CenterLoss kernel for Trainium2 (raw Bass/Bacc, no Tile), 8-core
data-parallel.

Key algebraic insight: the reference builds the full [B, C] squared-
distance matrix and masks it with one-hot(labels), so only
distmat[i, labels[i]] survives.  The loss is therefore

    loss = (1/B) * sum_i || x_i - centers[labels[i]] ||^2

which needs only a gather of each sample's center row (indirect DMA), not
the [4096, 10000] matmul.

Sharding: data-parallel over the batch.  Each of the 8 cores gets 512
samples (x shard + labels shard) and the full replicated centers table in
DRAM; it gathers its 512 center rows, computes
(sum ||x - c||^2) / B on device, and the host all-reduces (sums) the 8
partial scalars.

Per core (512 samples = 4 chunks x 128 partitions, interleaved layout:
chunk a holds samples {4p + a}, one per partition p):
  Sync   : labels DMA ([128,4] int32 tile, 16 B strips), then x as two
           DMAs with 4 KB contiguous per-partition strips, out DMA
  GpSimd : ones memset, 4 indirect gathers (offset AP = labels column a,
           one index per partition; one DMA sem lane per gather)
  Vector : per chunk subtract; final free-dim reduce of partials
  Scalar : per chunk Square activation w/ accum; final PSUM->SBUF copy
  Tensor : [1,1] = total.T @ (ones/B) partition reduction (the 1/B scale
           is folded into the ones vector)

Manual semaphores; no Tile exit drain+butterfly+sem-clear (the bass entry
preamble clears sems, so re-execution stays safe).
"""

from contextlib import ExitStack

import numpy as np

import concourse.bacc as bacc
import concourse.bass as bass
from concourse import mybir
from concourse.bass_utils import run_bass_kernel_spmd

BATCH = 4096
NUM_CLASSES = 10000
FEAT_DIM = 512
N_CORES = 8
BPC = BATCH // N_CORES   # samples per core = 512
P = 128                  # SBUF partitions
CHUNKS = BPC // P        # 4 chunks of 128 samples per core

AF = mybir.AluOpType

_NC_CACHE = {}


def _build_bass():
    nc = bacc.Bacc(None, target_bir_lowering=False)

    x_in = nc.dram_tensor("x", [BPC, FEAT_DIM], mybir.dt.float32,
                          kind="ExternalInput")
    lab_in = nc.dram_tensor("labels", [BPC], mybir.dt.int32,
                            kind="ExternalInput")
    cen_in = nc.dram_tensor("centers", [NUM_CLASSES, FEAT_DIM],
                            mybir.dt.float32, kind="ExternalInput")
    out_t = nc.dram_tensor("out", [1, 1], mybir.dt.float32,
                           kind="ExternalOutput")

    with ExitStack() as ctx:
        ec = ctx.enter_context
        lab_sb = ec(nc.sbuf_tensor("lab_sb", [P, CHUNKS], mybir.dt.int32))
        xt = ec(nc.sbuf_tensor("xt", [P, CHUNKS * FEAT_DIM],
                               mybir.dt.float32))
        ct = ec(nc.sbuf_tensor("ct", [P, CHUNKS * FEAT_DIM],
                               mybir.dt.float32))
        dds = [ec(nc.sbuf_tensor(f"dd{a}", [P, FEAT_DIM], mybir.dt.float32))
               for a in range(CHUNKS)]
        sqs = [ec(nc.sbuf_tensor(f"sq{a}", [P, FEAT_DIM], mybir.dt.float32))
               for a in range(CHUNKS)]
        partials = ec(nc.sbuf_tensor("partials", [P, CHUNKS],
                                     mybir.dt.float32))
        total = ec(nc.sbuf_tensor("total", [P, 1], mybir.dt.float32))
        ones = ec(nc.sbuf_tensor("ones", [P, 1], mybir.dt.float32))
        res = ec(nc.sbuf_tensor("res", [1, 1], mybir.dt.float32))
        ps = ec(nc.psum_tensor("ps", [1, 1], mybir.dt.float32))
        zidx = ec(nc.sbuf_tensor("zidx", [2, 1], mybir.dt.int32))
        warm = ec(nc.sbuf_tensor("warm", [2, FEAT_DIM], mybir.dt.float32))
        s_lab = ec(nc.semaphore("s_lab"))
        s_xs = [ec(nc.semaphore(f"s_x{a}")) for a in range(CHUNKS)]
        s_cts = [ec(nc.semaphore(f"s_ct{a}")) for a in range(CHUNKS)]
        s_g = ec(nc.semaphore("s_g"))
        s_wz = ec(nc.semaphore("s_wz"))
        s_warm = ec(nc.semaphore("s_warm"))
        s_sub = ec(nc.semaphore("s_sub"))
        s_acc = ec(nc.semaphore("s_acc"))
        s_v = ec(nc.semaphore("s_v"))
        s_pe = ec(nc.semaphore("s_pe"))
        s_res = ec(nc.semaphore("s_res"))
        s_out = ec(nc.semaphore("s_out"))

        # ---- Sync: labels first (gathers depend on them), then x as two
        # halves with 4 KB contiguous strips (partition p holds rows
        # 4p..4p+3; half h covers chunks {2h, 2h+1} = rows 4p+2h, 4p+2h+1).
        nc.scalar.dma_start(
            out=lab_sb[:],
            in_=lab_in[:].rearrange("(p a) -> p a", a=CHUNKS),
        ).then_inc(s_lab, 16)
        H = CHUNKS // 2
        for h in range(2):
            nc.sync.dma_start(
                out=xt[:, h * H * FEAT_DIM:(h + 1) * H * FEAT_DIM],
                in_=x_in[:].rearrange(
                    "(p h g) f -> p h (g f)", h=2, g=H)[:, h, :],
            ).then_inc(s_xs[h], 16)

        # ---- GpSimd: ones + warm-up gather (primes the SWDGE queue and
        # SDMA engines while waiting for labels, hiding the ~1.8us
        # doorbell-to-first-packet latency of the first real gather),
        # then the real gathers (SWDGE) ----
        nc.gpsimd.memset(ones[:], 1.0 / BATCH).then_inc(s_g, 1)
        nc.gpsimd.memset(zidx[:], 0).then_inc(s_wz, 1)
        nc.gpsimd.wait_ge(s_wz, 1)
        nc.gpsimd.indirect_dma_start(
            out=warm[:],
            out_offset=None,
            in_=cen_in[:],
            in_offset=bass.IndirectOffsetOnAxis(ap=zidx[:, 0:1], axis=0),
        ).then_inc(s_warm, 16)
        nc.gpsimd.wait_ge(s_lab, 16)
        for a in range(CHUNKS):
            nc.gpsimd.indirect_dma_start(
                out=ct[:, a * FEAT_DIM:(a + 1) * FEAT_DIM],
                out_offset=None,
                in_=cen_in[:],
                in_offset=bass.IndirectOffsetOnAxis(
                    ap=lab_sb[:, a:a + 1], axis=0),
            ).then_inc(s_cts[a], 16)

        # ---- Vector: per-chunk subtract ----
        for a in range(CHUNKS):
            sl = slice(a * FEAT_DIM, (a + 1) * FEAT_DIM)
            nc.vector.wait_ge(s_xs[a // (CHUNKS // 2)], 16)
            nc.vector.wait_ge(s_cts[a], 16)
            nc.vector.tensor_tensor(
                out=dds[a][:], in0=xt[:, sl], in1=ct[:, sl],
                op=AF.subtract).then_inc(s_sub, 1)

        # ---- Scalar: per-chunk square + accumulate along free dim ----
        for a in range(CHUNKS):
            nc.scalar.wait_ge(s_sub, a + 1)
            nc.scalar.activation(
                out=sqs[a][:], in_=dds[a][:],
                func=mybir.ActivationFunctionType.Square,
                accum_out=partials[:, a:a + 1]).then_inc(s_acc, 1)

        # ---- Vector: reduce partials over free dim ----
        nc.vector.wait_ge(s_acc, CHUNKS)
        nc.vector.reduce_sum(
            out=total[:], in_=partials[:],
            axis=mybir.AxisListType.X).then_inc(s_v, 1)

        # ---- Tensor: partition reduction (scale folded into ones) ----
        nc.tensor.wait_ge(s_v, 1)
        nc.tensor.wait_ge(s_g, 1)
        nc.tensor.matmul(out=ps[:], lhsT=total[:], rhs=ones[:],
                         start=True, stop=True).then_inc(s_pe, 1)

        # ---- Scalar: PSUM -> SBUF ----
        nc.scalar.wait_ge(s_pe, 1)
        nc.scalar.copy(res[:], ps[:]).then_inc(s_res, 1)

        # ---- Sync: output DMA, wait for completion before halt ----
        nc.sync.wait_ge(s_res, 1)
        nc.sync.dma_start(out=out_t[:], in_=res[:],
                          single_packet=True).then_inc(s_out, 16)
        nc.sync.wait_ge(s_out, 16)

    # Bacc defers register allocation + event-semaphore splitting to
    # compile(); the pjrt exec path serializes without calling it.
    nc.compile()
    return nc


def get_nc():
    if "nc" not in _NC_CACHE:
        _NC_CACHE["nc"] = _build_bass()
    return _NC_CACHE["nc"]


def kernel(x, labels, centers, _run_kwargs=None):
    x = np.ascontiguousarray(x, dtype=np.float32)
    labels = np.ascontiguousarray(labels).astype(np.int32)
    centers = np.ascontiguousarray(centers, dtype=np.float32)

    nc = get_nc()
    in_maps = [
        {
            "x": x[c * BPC:(c + 1) * BPC],
            "labels": labels[c * BPC:(c + 1) * BPC],
            "centers": centers,
        }
        for c in range(N_CORES)
    ]
    kwargs = _run_kwargs or {}
    out = run_bass_kernel_spmd(nc, in_maps, core_ids=list(range(N_CORES)),
                               **kwargs)
    # all-reduce the 8 per-core partial scalars (each already / BATCH)
    total = np.float32(0.0)
    for r in out.results:
        total = total + np.float32(r["out"][0, 0])
    if kwargs:
        kernel.last_run = out
    return np.asarray(total, dtype=np.float32)


# revision 15
# speedup vs baseline: 1.0828x; 1.0828x over previous
"""CenterLoss kernel for Trainium2 (raw Bass/Bacc, no Tile), 8-core
data-parallel.

Key algebraic insight: the reference builds the full [B, C] squared-
distance matrix and masks it with one-hot(labels), so only
distmat[i, labels[i]] survives.  The loss is therefore

    loss = (1/B) * sum_i || x_i - centers[labels[i]] ||^2

which needs only a gather of each sample's center row (indirect DMA), not
the [4096, 10000] matmul.

Sharding: data-parallel over the batch.  Each of the 8 cores gets 512
samples (x shard + labels shard) and the full replicated centers table in
DRAM; it gathers its 512 center rows, computes
(sum ||x - c||^2) / B on device, and the host all-reduces (sums) the 8
partial scalars.

Per core (512 samples = 4 chunks x 128 partitions, interleaved layout:
chunk a holds samples {4p + a}, one per partition p):
  Sync   : labels DMA ([128,4] int32 tile, 16 B strips), then x as two
           DMAs with 4 KB contiguous per-partition strips, out DMA
  GpSimd : ones memset, 4 indirect gathers (offset AP = labels column a,
           one index per partition; one DMA sem lane per gather)
  Vector : per chunk subtract; final free-dim reduce of partials
  Scalar : per chunk Square activation w/ accum; final PSUM->SBUF copy
  Tensor : [1,1] = total.T @ (ones/B) partition reduction (the 1/B scale
           is folded into the ones vector)

Manual semaphores; no Tile exit drain+butterfly+sem-clear (the bass entry
preamble clears sems, so re-execution stays safe).
"""

from contextlib import ExitStack

import numpy as np

import concourse.bacc as bacc
import concourse.bass as bass
from concourse import mybir
from concourse.bass_utils import run_bass_kernel_spmd

BATCH = 4096
NUM_CLASSES = 10000
FEAT_DIM = 512
N_CORES = 8
BPC = BATCH // N_CORES   # samples per core = 512
P = 128                  # SBUF partitions
CHUNKS = BPC // P        # 4 chunks of 128 samples per core

AF = mybir.AluOpType

_NC_CACHE = {}


def _build_bass():
    nc = bacc.Bacc(None, target_bir_lowering=False)

    x_in = nc.dram_tensor("x", [BPC, FEAT_DIM], mybir.dt.float32,
                          kind="ExternalInput")
    lab_in = nc.dram_tensor("labels", [BPC], mybir.dt.int32,
                            kind="ExternalInput")
    cen_in = nc.dram_tensor("centers", [NUM_CLASSES, FEAT_DIM],
                            mybir.dt.float32, kind="ExternalInput")
    out_t = nc.dram_tensor("out", [1, 1], mybir.dt.float32,
                           kind="ExternalOutput")

    with ExitStack() as ctx:
        ec = ctx.enter_context
        lab_sb = ec(nc.sbuf_tensor("lab_sb", [P, CHUNKS], mybir.dt.int32))
        xt = ec(nc.sbuf_tensor("xt", [P, CHUNKS * FEAT_DIM],
                               mybir.dt.float32))
        ct = ec(nc.sbuf_tensor("ct", [P, CHUNKS * FEAT_DIM],
                               mybir.dt.float32))
        dds = [ec(nc.sbuf_tensor(f"dd{a}", [P, FEAT_DIM], mybir.dt.float32))
               for a in range(CHUNKS)]
        sqs = [ec(nc.sbuf_tensor(f"sq{a}", [P, FEAT_DIM], mybir.dt.float32))
               for a in range(CHUNKS)]
        partials = ec(nc.sbuf_tensor("partials", [P, CHUNKS],
                                     mybir.dt.float32))
        total = ec(nc.sbuf_tensor("total", [P, 1], mybir.dt.float32))
        ones = ec(nc.sbuf_tensor("ones", [P, 1], mybir.dt.float32))
        res = ec(nc.sbuf_tensor("res", [1, 1], mybir.dt.float32))
        ps = ec(nc.psum_tensor("ps", [1, 1], mybir.dt.float32))
        zidx = ec(nc.sbuf_tensor("zidx", [32, 1], mybir.dt.int32))
        warm = ec(nc.sbuf_tensor("warm", [32, FEAT_DIM], mybir.dt.float32))
        s_lab = ec(nc.semaphore("s_lab"))
        s_xs = [ec(nc.semaphore(f"s_x{a}")) for a in range(CHUNKS)]
        s_cts = [ec(nc.semaphore(f"s_ct{a}")) for a in range(CHUNKS)]
        s_g = ec(nc.semaphore("s_g"))
        s_wz = ec(nc.semaphore("s_wz"))
        s_warm = ec(nc.semaphore("s_warm"))
        s_sub = ec(nc.semaphore("s_sub"))
        s_acc = ec(nc.semaphore("s_acc"))
        s_v = ec(nc.semaphore("s_v"))
        s_pe = ec(nc.semaphore("s_pe"))
        s_res = ec(nc.semaphore("s_res"))
        s_out = ec(nc.semaphore("s_out"))

        # ---- Sync: labels first (gathers depend on them), then x as two
        # halves with 4 KB contiguous strips (partition p holds rows
        # 4p..4p+3; half h covers chunks {2h, 2h+1} = rows 4p+2h, 4p+2h+1).
        nc.sync.dma_start(
            out=lab_sb[:],
            in_=lab_in[:].rearrange("(p a) -> p a", a=CHUNKS),
        ).then_inc(s_lab, 16)
        H = CHUNKS // 2
        for h in range(2):
            nc.sync.dma_start(
                out=xt[:, h * H * FEAT_DIM:(h + 1) * H * FEAT_DIM],
                in_=x_in[:].rearrange(
                    "(p h g) f -> p h (g f)", h=2, g=H)[:, h, :],
            ).then_inc(s_xs[h], 16)

        # ---- GpSimd: ones + gathers (SWDGE) ----
        nc.gpsimd.memset(ones[:], 1.0 / BATCH).then_inc(s_g, 1)
        nc.gpsimd.wait_ge(s_lab, 16)
        for a in range(CHUNKS):
            nc.gpsimd.indirect_dma_start(
                out=ct[:, a * FEAT_DIM:(a + 1) * FEAT_DIM],
                out_offset=None,
                in_=cen_in[:],
                in_offset=bass.IndirectOffsetOnAxis(
                    ap=lab_sb[:, a:a + 1], axis=0),
            ).then_inc(s_cts[a], 16)

        # ---- Vector: per-chunk subtract ----
        for a in range(CHUNKS):
            sl = slice(a * FEAT_DIM, (a + 1) * FEAT_DIM)
            nc.vector.wait_ge(s_xs[a // (CHUNKS // 2)], 16)
            nc.vector.wait_ge(s_cts[a], 16)
            nc.vector.tensor_tensor(
                out=dds[a][:], in0=xt[:, sl], in1=ct[:, sl],
                op=AF.subtract).then_inc(s_sub, 1)

        # ---- Scalar: per-chunk square + accumulate along free dim ----
        for a in range(CHUNKS):
            nc.scalar.wait_ge(s_sub, a + 1)
            nc.scalar.activation(
                out=sqs[a][:], in_=dds[a][:],
                func=mybir.ActivationFunctionType.Square,
                accum_out=partials[:, a:a + 1]).then_inc(s_acc, 1)

        # ---- Vector: reduce partials over free dim ----
        nc.vector.wait_ge(s_acc, CHUNKS)
        nc.vector.reduce_sum(
            out=total[:], in_=partials[:],
            axis=mybir.AxisListType.X).then_inc(s_v, 1)

        # ---- Tensor: partition reduction (scale folded into ones) ----
        nc.tensor.wait_ge(s_v, 1)
        nc.tensor.wait_ge(s_g, 1)
        nc.tensor.matmul(out=ps[:], lhsT=total[:], rhs=ones[:],
                         start=True, stop=True).then_inc(s_pe, 1)

        # ---- Scalar: PSUM -> SBUF ----
        nc.scalar.wait_ge(s_pe, 1)
        nc.scalar.copy(res[:], ps[:]).then_inc(s_res, 1)

        # ---- Sync: output DMA, wait for completion before halt ----
        nc.sync.wait_ge(s_res, 1)
        nc.sync.dma_start(out=out_t[:], in_=res[:]).then_inc(s_out, 16)
        nc.sync.wait_ge(s_out, 16)

    # Bacc defers register allocation + event-semaphore splitting to
    # compile(); the pjrt exec path serializes without calling it.
    nc.compile()
    return nc


def get_nc():
    if "nc" not in _NC_CACHE:
        _NC_CACHE["nc"] = _build_bass()
    return _NC_CACHE["nc"]


def kernel(x, labels, centers, _run_kwargs=None):
    x = np.ascontiguousarray(x, dtype=np.float32)
    labels = np.ascontiguousarray(labels).astype(np.int32)
    centers = np.ascontiguousarray(centers, dtype=np.float32)

    nc = get_nc()
    in_maps = [
        {
            "x": x[c * BPC:(c + 1) * BPC],
            "labels": labels[c * BPC:(c + 1) * BPC],
            "centers": centers,
        }
        for c in range(N_CORES)
    ]
    kwargs = _run_kwargs or {}
    out = run_bass_kernel_spmd(nc, in_maps, core_ids=list(range(N_CORES)),
                               **kwargs)
    # all-reduce the 8 per-core partial scalars (each already / BATCH)
    total = np.float32(0.0)
    for r in out.results:
        total = total + np.float32(r["out"][0, 0])
    if kwargs:
        kernel.last_run = out
    return np.asarray(total, dtype=np.float32)
